# revision 47
# baseline (speedup 1.0000x reference)
"""Trainium2 Bass kernel for nn_BPPSLodeModel (moe_routing).

Model (per reference):
  f_ps = LayerNorm(x_ps) * gamma_ps + beta_ps        # [N, 512]
  f_mp = LayerNorm(x_mp) * gamma_mp + beta_mp        # [N, 256]
  e_ps = species_mlp(f_ps, W_ps1/2/3)                # [N, 1], 4 contiguous species blocks
  e_mp = species_mlp(f_mp, W_mp1/2/3)                # [N, 1]
  out  = segment_sum(e_ps + e_mp, batch, 512)        # [512, 1]

Sharding: data-parallel over atoms. 65536 atoms / 8 cores = 8192 atoms per
core; species blocks are 16384 atoms, so core c holds species c//2 only and
receives just that species' weights. gamma/beta are folded into W1/b1 on the
host (exact: LN(x)*g+b @ W1 == LN(x) @ (g*W1) + b@W1). Each core computes a
partial [512] per-molecule energy vector over its atoms; the host sums the 8
partials (the gather step of the data-parallel reduction).

On-core dataflow (all matmuls in float32r, fp32 PSUM accumulate):
  DMA x (atom-major) -> bn_stats/bn_aggr (DVE) -> rstd,t (small ops)
  -> normalize on ScalarE (x*rstd + t, rounds to f32r)
  -> PE transpose to feature-major X^T -> DVE copy PSUM->SBUF
  -> L1/L2 matmuls (weights stationary) + SiLU(+bias) on ScalarE
  -> L3 (M=1) -> per-atom energy row e[1, atoms]
  -> e reshaped to [128, chunk] columns via DRAM bounce
  -> segment-sum: one-hot(batch)==iota matmuls accumulating into PSUM [1,512]
"""

import numpy as np

N_ATOMS = 65536
N_CORES = 8
APC = N_ATOMS // N_CORES        # atoms per core = 8192
N_SPECIES = 4
N_MOL = 512
F_PS = 512
F_MP = 256
H = 256
EPS = 1e-5
SUP = 512                       # atoms per supertile
NSUP = APC // SUP               # 16
P = 128
NCHUNK = APC // P               # 64 segment-sum chunks
NEWTON_ITERS = 2                # rsqrt Newton refinements (rel err ~5e-6)
SEG_WIN = 256                   # molecule window per chunk (atoms host-sorted)


def _seg_starts():
    # Atoms are host-sorted by molecule and shifted to local coords with a
    # +64 guard: chunk ch's atoms sit near local molecule 4*ch + 64, with
    # statistical drift well under +-40.  A 256-wide window is bulletproof.
    # Chunk 0 uses the full 512 so its start=True matmul claims and writes
    # the whole PSUM bank (idempotent across executions, sim-exact).
    sw = [(min(max(4 * ch - 60, 0), N_MOL - SEG_WIN), SEG_WIN)
          for ch in range(NCHUNK)]
    sw[0] = (0, N_MOL)
    return sw

_CACHE = {}


def _src_tag():
    import hashlib
    try:
        with open(__file__, "rb") as f:
            return int(hashlib.md5(f.read()).hexdigest()[:4], 16) % 509 + 2
    except OSError:
        return 1


def _build(nrep=1):
    if ("nc", nrep) in _CACHE:
        return _CACHE[("nc", nrep)]

    import concourse.bacc as bacc
    import concourse.tile as tile
    from concourse import mybir

    F32 = mybir.dt.float32
    F32R = mybir.dt.float32r
    ACTF = mybir.ActivationFunctionType

    nc = bacc.Bacc("TRN2", target_bir_lowering=False, debug=False,
                   num_devices=N_CORES)

    xps_d = nc.dram_tensor("xps", [APC, F_PS], F32, kind="ExternalInput")
    xmp_d = nc.dram_tensor("xmp", [APC, F_MP], F32, kind="ExternalInput")
    bcol_d = nc.dram_tensor("bcol", [P, NCHUNK], F32, kind="ExternalInput")
    wps1_d = nc.dram_tensor("wps1", [F_PS, H], F32, kind="ExternalInput")
    bps1_d = nc.dram_tensor("bps1", [P, H // P], F32, kind="ExternalInput")
    wps2_d = nc.dram_tensor("wps2", [H, H], F32, kind="ExternalInput")
    wps3_d = nc.dram_tensor("wps3", [P, H // P], F32, kind="ExternalInput")
    wmp1_d = nc.dram_tensor("wmp1", [F_MP, H], F32, kind="ExternalInput")
    bmp1_d = nc.dram_tensor("bmp1", [P, H // P], F32, kind="ExternalInput")
    wmp2_d = nc.dram_tensor("wmp2", [H, H], F32, kind="ExternalInput")
    wmp3_d = nc.dram_tensor("wmp3", [P, H // P], F32, kind="ExternalInput")
    out_d = nc.dram_tensor("out", [N_MOL], F32, kind="ExternalOutput")
    # NEFF/jit caches key on the I/O signature, not the kernel body: encode
    # (source hash, nrep) into a dummy input's shape so edits recompile.
    cb_d = nc.dram_tensor("cachebust", [nrep, _src_tag()], F32,
                          kind="ExternalInput")

    from contextlib import ExitStack
    with tile.TileContext(nc) as tc, ExitStack() as ctx:
        consts = ctx.enter_context(tc.tile_pool(name="consts", bufs=1))
        xpool = ctx.enter_context(tc.tile_pool(name="x", bufs=5))
        xnpool = ctx.enter_context(tc.tile_pool(name="xn", bufs=8))
        xtpool = ctx.enter_context(tc.tile_pool(name="xt", bufs=8))
        hpool = ctx.enter_context(tc.tile_pool(name="h", bufs=6))
        stats = ctx.enter_context(tc.tile_pool(name="stats", bufs=16))
        onepool = ctx.enter_context(tc.tile_pool(name="onehot", bufs=4))
        erow_pool = ctx.enter_context(tc.tile_pool(name="erow", bufs=2))
        dram = ctx.enter_context(tc.tile_pool(name="dram", bufs=2, space="DRAM"))
        ps_tp = ctx.enter_context(tc.tile_pool(name="ps_tp", bufs=2, space="PSUM"))
        ps_mm = ctx.enter_context(tc.tile_pool(name="ps_mm", bufs=3, space="PSUM"))
        ps_e = ctx.enter_context(tc.tile_pool(name="ps_e", bufs=2, space="PSUM"))
        ps_sg = ctx.enter_context(tc.tile_pool(name="ps_sg", bufs=1, space="PSUM"))

        # ---- constants -------------------------------------------------
        ident_f = consts.tile([P, P], F32)
        nc.gpsimd.memset(ident_f[:], 0.0)
        nc.gpsimd.affine_select(
            out=ident_f[:], in_=ident_f[:],
            compare_op=mybir.AluOpType.not_equal, fill=1.0,
            base=0, pattern=[[-1, P]], channel_multiplier=1,
        )
        ident = consts.tile([P, P], F32R)
        nc.vector.tensor_copy(out=ident[:], in_=ident_f[:])
        iota_t = consts.tile([P, N_MOL], F32)
        nc.gpsimd.iota(iota_t[:], pattern=[[1, N_MOL]], base=0,
                       channel_multiplier=0,
                       allow_small_or_imprecise_dtypes=True)
        bcol_t = consts.tile([P, NCHUNK], F32)
        nc.sync.dma_start(out=bcol_t[:], in_=bcol_d[:])
        cb_t = consts.tile([nrep, _src_tag()], F32)
        nc.sync.dma_start(out=cb_t[:], in_=cb_d[:])

        # weights: sync-DMA fp32, round to f32r on ScalarE (Copy shares the
        # Silu table set, and ACT is idle during warmup)
        wtmp_pool = ctx.enter_context(tc.tile_pool(name="wtmp", bufs=2))

        def wload(dram_t, kparts, n):
            tiles = []
            for kt in range(kparts):
                tmp = wtmp_pool.tile([P, n], F32, tag="wtmp")
                nc.sync.dma_start(out=tmp[:], in_=dram_t[kt * P:(kt + 1) * P, :])
                t = consts.tile([P, n], F32R, tag=f"w_{dram_t.name}_{kt}")
                nc.scalar.copy(out=t[:], in_=tmp[:])
                tiles.append(t)
            return tiles

        wps1_t = wload(wps1_d, F_PS // P, H)
        wps2_t = wload(wps2_d, H // P, H)
        wmp1_t = wload(wmp1_d, F_MP // P, H)
        wmp2_t = wload(wmp2_d, H // P, H)
        wps3_t = wload(wps3_d, 1, H // P)[0]
        wmp3_t = wload(wmp3_d, 1, H // P)[0]
        bps1_t = consts.tile([P, H // P], F32)
        nc.sync.dma_start(out=bps1_t[:], in_=bps1_d[:])
        bmp1_t = consts.tile([P, H // P], F32)
        nc.sync.dma_start(out=bmp1_t[:], in_=bmp1_d[:])

        # ---- main streams ---------------------------------------------
        def stream_sup(x_d, F, w1_t, b1_t, w2_t, w3_t, s, pe,
                       first_l3, last_l3, x3_pre=None):
            KT1 = F // P       # k-tiles of layer 1
            AT = SUP // P      # atom sub-tiles per supertile
            if True:
                a0 = s * SUP
                if x3_pre is not None:
                    x3 = x3_pre
                else:
                    x3 = xpool.tile([P, AT, F], F32, tag="x")
                    nc.sync.dma_start(
                        out=x3[:],
                        in_=x_d[a0:a0 + SUP, :].rearrange(
                            "(at p) f -> p at f", p=P),
                    )
                mv = stats.tile([P, AT, 2], F32, tag="mv")
                for at in range(AT):
                    st6 = stats.tile([P, 6], F32, tag="st6")
                    nc.vector.bn_stats(out=st6[:], in_=x3[:, at, :])
                    nc.vector.bn_aggr(out=mv[:, at, :], in_=st6[:])
                # rstd = rsqrt(var+eps) via bit-trick + 2 Newton steps (DVE)
                veps = stats.tile([P, AT], F32, tag="veps")
                nc.vector.tensor_scalar_add(veps[:], mv[:, :, 1], EPS)
                yi = stats.tile([P, AT], mybir.dt.int32, tag="yi")
                nc.vector.tensor_scalar(
                    out=yi[:], in0=veps[:].bitcast(mybir.dt.int32),
                    scalar1=1, scalar2=None,
                    op0=mybir.AluOpType.logical_shift_right)
                nc.vector.tensor_scalar(
                    out=yi[:], in0=yi[:],
                    scalar1=0x5F3759DF, scalar2=-1,
                    op0=mybir.AluOpType.subtract,
                    op1=mybir.AluOpType.mult)
                y = yi[:].bitcast(F32)
                tmp = stats.tile([P, AT], F32, tag="tmp")
                for _ in range(NEWTON_ITERS):
                    nc.vector.tensor_mul(tmp[:], veps[:], y)
                    nc.vector.tensor_mul(tmp[:], tmp[:], y)
                    nc.vector.tensor_scalar(
                        out=tmp[:], in0=tmp[:], scalar1=-0.5, scalar2=1.5,
                        op0=mybir.AluOpType.mult, op1=mybir.AluOpType.add)
                    nc.vector.tensor_mul(yi[:].bitcast(F32), y, tmp[:])
                rstd = yi[:].bitcast(F32)
                tsh = stats.tile([P, AT], F32, tag="tsh")
                nc.vector.tensor_scalar_mul(tsh[:], mv[:, :, 0], -1.0)
                nc.vector.tensor_mul(tsh[:], tsh[:], rstd)
                xn = []
                for at in range(AT):
                    xnt = xnpool.tile([P, F], F32R, tag="xn")
                    nc.gpsimd.tensor_scalar(
                        out=xnt[:], in0=x3[:, at, :],
                        scalar1=rstd[:, at:at + 1],
                        scalar2=tsh[:, at:at + 1],
                        op0=mybir.AluOpType.mult,
                        op1=mybir.AluOpType.add)
                    xn.append(xnt)
                xt = []
                for ft in range(KT1):
                    pxt = ps_tp.tile([P, SUP], F32R, space="PSUM", tag="pxt")
                    for at in range(AT):
                        nc.tensor.matmul(
                            out=pxt[:, at * P:(at + 1) * P],
                            lhsT=xn[at][:, ft * P:(ft + 1) * P],
                            rhs=ident[:],
                            is_transpose=True,
                            start=(at == 0), stop=(at == AT - 1),
                        )
                    xtt = xtpool.tile([P, SUP], F32R, tag="xt")
                    if F == F_PS:
                        nc.scalar.copy(out=xtt[:], in_=pxt[:])
                    else:
                        nc.vector.tensor_copy(out=xtt[:], in_=pxt[:])
                    xt.append(xtt)
                h1 = []
                for mt in range(H // P):
                    pg = ps_mm.tile([P, SUP], F32, space="PSUM", tag="pg")
                    for kt in range(KT1):
                        nc.tensor.matmul(
                            out=pg[:],
                            lhsT=w1_t[kt][:, mt * P:(mt + 1) * P],
                            rhs=xt[kt][:],
                            start=(kt == 0), stop=(kt == KT1 - 1),
                        )
                    h1t = hpool.tile([P, SUP], F32R, tag="h1")
                    nc.scalar.activation(out=h1t[:], in_=pg[:],
                                         func=ACTF.Silu,
                                         bias=b1_t[:, mt:mt + 1], scale=1.0)
                    h1.append(h1t)
                h2 = []
                for mt in range(H // P):
                    pg = ps_mm.tile([P, SUP], F32, space="PSUM", tag="pg")
                    for kt in range(H // P):
                        nc.tensor.matmul(
                            out=pg[:],
                            lhsT=w2_t[kt][:, mt * P:(mt + 1) * P],
                            rhs=h1[kt][:],
                            start=(kt == 0), stop=(kt == H // P - 1),
                        )
                    h2t = hpool.tile([P, SUP], F32R, tag="h2")
                    nc.scalar.activation(out=h2t[:], in_=pg[:],
                                         func=ACTF.Silu, scale=1.0)
                    h2.append(h2t)
                for kt in range(H // P):
                    nc.tensor.matmul(
                        out=pe[:],
                        lhsT=w3_t[:, kt:kt + 1],
                        rhs=h2[kt][:],
                        start=(first_l3 and kt == 0),
                        stop=(last_l3 and kt == H // P - 1),
                    )

        CPS = SUP // P  # segsum chunks per supertile
        seg_starts = _seg_starts()

        for _rep in range(nrep):
            ps_seg = ps_sg.tile([1, N_MOL], F32, space="PSUM", tag="pseg")
            for s in range(NSUP):
                # e_ps and e_mp L3s accumulate into one PSUM tile [1, SUP]
                pe = ps_e.tile([1, SUP], F32, space="PSUM", tag="pe")
                stream_sup(xps_d, F_PS, wps1_t, bps1_t, wps2_t, wps3_t,
                           s, pe, True, False)
                stream_sup(xmp_d, F_MP, wmp1_t, bmp1_t, wmp2_t, wmp3_t,
                           s, pe, False, True)
                e_sb = erow_pool.tile([1, SUP], F32, tag="e_sb")
                nc.vector.tensor_copy(out=e_sb[:], in_=pe[:])
                e_dram = dram.tile([SUP], F32, tag="e_dram")
                nc.sync.dma_start(out=e_dram[:], in_=e_sb[:])
                e_cols = erow_pool.tile([P, CPS], mybir.dt.float32r,
                                        tag="e_cols")
                nc.gpsimd.dma_start(
                    out=e_cols[:],
                    in_=e_dram[:].rearrange("(c p) -> p c", p=P),
                )
                for cc in range(CPS):
                    ch = s * CPS + cc
                    st, wid = seg_starts[ch]
                    oh = onepool.tile([P, N_MOL], mybir.dt.float32r,
                                      tag="oh")
                    oh_eng = nc.vector if ch % 2 == 0 else nc.gpsimd
                    oh_eng.tensor_scalar(
                        out=oh[:, :wid], in0=iota_t[:, :wid],
                        scalar1=bcol_t[:, ch:ch + 1],
                        scalar2=None, op0=mybir.AluOpType.is_equal,
                    )
                    nc.tensor.matmul(
                        out=ps_seg[0:1, st:st + wid],
                        lhsT=e_cols[:, cc:cc + 1], rhs=oh[:, :wid],
                        start=(ch == 0), stop=(ch == NCHUNK - 1),
                    )
            out_sb = erow_pool.tile([1, N_MOL], F32, tag="out_sb")
            nc.vector.tensor_copy(out=out_sb[:], in_=ps_seg[:])
            nc.sync.dma_start(out=out_d[:], in_=out_sb[:])

    nc.compile()
    _CACHE[("nc", nrep)] = nc
    return nc


def _shard_inputs(x_ps, x_mp, batch, gamma_ps, beta_ps, gamma_mp, beta_mp,
                  W_ps1, W_ps2, W_ps3, W_mp1, W_mp2, W_mp3, nrep=1):
    f32 = np.float32
    cachebust = np.zeros((nrep, _src_tag()), f32)
    x_ps = np.asarray(x_ps, dtype=f32)
    x_mp = np.asarray(x_mp, dtype=f32)
    batch = np.asarray(batch).astype(np.int64)
    sw = _seg_starts()
    starts = np.array([s for s, _ in sw])
    widths = np.array([w for _, w in sw])
    blk = N_ATOMS // N_SPECIES
    in_maps = []
    for c in range(N_CORES):
        s = c // (N_CORES // N_SPECIES)
        h = c % (N_CORES // N_SPECIES)
        bb = batch[s * blk:(s + 1) * blk]
        perm = np.argsort(bb, kind="stable")[h * APC:(h + 1) * APC]
        gidx = s * blk + perm
        bs = bb[perm]
        # local molecule coords: shift by -256*h +64 guard, then per-chunk
        # window start subtraction (window membership asserted below)
        shifted = bs - (N_MOL // 2) * h + 64
        bc = shifted.reshape(NCHUNK, P) - starts[:, None]
        assert (bc >= 0).all() and (bc < widths[:, None]).all(), \
            "segment window overflow - pathological batch distribution"
        w1p = (np.asarray(gamma_ps, f32)[:, None] * np.asarray(W_ps1[s], f32))
        b1p = (np.asarray(beta_ps, f32) @ np.asarray(W_ps1[s], f32))
        w1m = (np.asarray(gamma_mp, f32)[:, None] * np.asarray(W_mp1[s], f32))
        b1m = (np.asarray(beta_mp, f32) @ np.asarray(W_mp1[s], f32))
        in_maps.append({
            "cachebust": cachebust,
            "xps": np.ascontiguousarray(x_ps[gidx]),
            "xmp": np.ascontiguousarray(x_mp[gidx]),
            "bcol": np.ascontiguousarray(bc.T.astype(f32)),
            "wps1": np.ascontiguousarray(w1p.astype(f32)),
            "bps1": np.ascontiguousarray(b1p.astype(f32).reshape(H // P, P).T),
            "wps2": np.ascontiguousarray(np.asarray(W_ps2[s], dtype=f32)),
            "wps3": np.ascontiguousarray(
                np.asarray(W_ps3[s], dtype=f32)[:, 0].reshape(H // P, P).T),
            "wmp1": np.ascontiguousarray(w1m.astype(f32)),
            "bmp1": np.ascontiguousarray(b1m.astype(f32).reshape(H // P, P).T),
            "wmp2": np.ascontiguousarray(np.asarray(W_mp2[s], dtype=f32)),
            "wmp3": np.ascontiguousarray(
                np.asarray(W_mp3[s], dtype=f32)[:, 0].reshape(H // P, P).T),
        })
    return in_maps


def _gather_output(partials):
    """Sum per-core partial energies, undoing each core's local molecule
    coordinate shift (local j corresponds to global m = j + 256*h - 64)."""
    full = np.zeros(N_MOL, dtype=np.float64)
    for c, part in enumerate(partials):
        h = c % (N_CORES // N_SPECIES)
        off = (N_MOL // 2) * h - 64
        j = np.arange(N_MOL)
        m = j + off
        valid = (m >= 0) & (m < N_MOL)
        np.add.at(full, m[valid], part.astype(np.float64)[valid])
    return full.astype(np.float32)


def kernel(x_ps, x_mp, batch, gamma_ps, beta_ps, gamma_mp, beta_mp,
           W_ps1, W_ps2, W_ps3, W_mp1, W_mp2, W_mp3, _want_results=False):
    from concourse.bass_utils import run_bass_kernel_spmd

    nc = _build()
    in_maps = _shard_inputs(
        x_ps, x_mp, batch, gamma_ps, beta_ps, gamma_mp, beta_mp,
        W_ps1, W_ps2, W_ps3, W_mp1, W_mp2, W_mp3)
    res = run_bass_kernel_spmd(nc, in_maps, list(range(N_CORES)))
    partials = [res.results[c]["out"] for c in range(N_CORES)]
    out = _gather_output(partials).reshape(N_MOL, 1)
    if _want_results:
        return out, res
    return out


# revision 48
# speedup vs baseline: 1.6895x; 1.6895x over previous
"""Trainium2 Bass kernel for nn_BPPSLodeModel (moe_routing).

Model (per reference):
  f_ps = LayerNorm(x_ps) * gamma_ps + beta_ps        # [N, 512]
  f_mp = LayerNorm(x_mp) * gamma_mp + beta_mp        # [N, 256]
  e_ps = species_mlp(f_ps, W_ps1/2/3)                # [N, 1], 4 contiguous species blocks
  e_mp = species_mlp(f_mp, W_mp1/2/3)                # [N, 1]
  out  = segment_sum(e_ps + e_mp, batch, 512)        # [512, 1]

Sharding: data-parallel over atoms. 65536 atoms / 8 cores = 8192 atoms per
core; species blocks are 16384 atoms, so core c holds species c//2 only and
receives just that species' weights. gamma/beta are folded into W1/b1 on the
host (exact: LN(x)*g+b @ W1 == LN(x) @ (g*W1) + b@W1). Each core computes a
partial [512] per-molecule energy vector over its atoms; the host sums the 8
partials (the gather step of the data-parallel reduction).

On-core dataflow (all matmuls in float32r, fp32 PSUM accumulate):
  DMA x (atom-major) -> bn_stats/bn_aggr (DVE) -> rstd,t (small ops)
  -> normalize on ScalarE (x*rstd + t, rounds to f32r)
  -> PE transpose to feature-major X^T -> DVE copy PSUM->SBUF
  -> L1/L2 matmuls (weights stationary) + SiLU(+bias) on ScalarE
  -> L3 (M=1) -> per-atom energy row e[1, atoms]
  -> e reshaped to [128, chunk] columns via DRAM bounce
  -> segment-sum: one-hot(batch)==iota matmuls accumulating into PSUM [1,512]
"""

import numpy as np

N_ATOMS = 65536
N_CORES = 8
APC = N_ATOMS // N_CORES        # atoms per core = 8192
N_SPECIES = 4
N_MOL = 512
F_PS = 512
F_MP = 256
H = 256
EPS = 1e-5
SUP = 512                       # atoms per supertile
NSUP = APC // SUP               # 16
P = 128
NCHUNK = APC // P               # 64 segment-sum chunks
NEWTON_ITERS = 2                # rsqrt Newton refinements (rel err ~5e-6)
SEG_WIN = 256                   # molecule window per chunk (atoms host-sorted)


def _seg_starts():
    # Atoms are host-sorted by molecule and shifted to local coords with a
    # +64 guard: chunk ch's atoms sit near local molecule 4*ch + 64, with
    # statistical drift well under +-40.  A 256-wide window is bulletproof.
    # Chunk 0 uses the full 512 so its start=True matmul claims and writes
    # the whole PSUM bank (idempotent across executions, sim-exact).
    sw = [(min(max(4 * ch - 60, 0), N_MOL - SEG_WIN), SEG_WIN)
          for ch in range(NCHUNK)]
    sw[0] = (0, N_MOL)
    return sw

_CACHE = {}


def _src_tag():
    import hashlib
    try:
        with open(__file__, "rb") as f:
            return int(hashlib.md5(f.read()).hexdigest()[:4], 16) % 509 + 2
    except OSError:
        return 1


def _build(nrep=1):
    if ("nc", nrep) in _CACHE:
        return _CACHE[("nc", nrep)]

    import concourse.bacc as bacc
    import concourse.tile as tile
    from concourse import mybir

    F32 = mybir.dt.float32
    F32R = mybir.dt.float32r
    ACTF = mybir.ActivationFunctionType

    nc = bacc.Bacc("TRN2", target_bir_lowering=False, debug=False,
                   num_devices=N_CORES)

    xps_d = nc.dram_tensor("xps", [APC, F_PS], F32, kind="ExternalInput")
    xmp_d = nc.dram_tensor("xmp", [APC, F_MP], F32, kind="ExternalInput")
    bcol_d = nc.dram_tensor("bcol", [P, NCHUNK], F32, kind="ExternalInput")
    wps1_d = nc.dram_tensor("wps1", [F_PS, H], F32, kind="ExternalInput")
    bps1_d = nc.dram_tensor("bps1", [P, H // P], F32, kind="ExternalInput")
    wps2_d = nc.dram_tensor("wps2", [H, H], F32, kind="ExternalInput")
    wps3_d = nc.dram_tensor("wps3", [P, H // P], F32, kind="ExternalInput")
    wmp1_d = nc.dram_tensor("wmp1", [F_MP, H], F32, kind="ExternalInput")
    bmp1_d = nc.dram_tensor("bmp1", [P, H // P], F32, kind="ExternalInput")
    wmp2_d = nc.dram_tensor("wmp2", [H, H], F32, kind="ExternalInput")
    wmp3_d = nc.dram_tensor("wmp3", [P, H // P], F32, kind="ExternalInput")
    out_d = nc.dram_tensor("out", [N_MOL], F32, kind="ExternalOutput")
    # NEFF/jit caches key on the I/O signature, not the kernel body: encode
    # (source hash, nrep) into a dummy input's shape so edits recompile.
    cb_d = nc.dram_tensor("cachebust", [nrep, _src_tag()], F32,
                          kind="ExternalInput")

    from contextlib import ExitStack
    with tile.TileContext(nc) as tc, ExitStack() as ctx:
        consts = ctx.enter_context(tc.tile_pool(name="consts", bufs=1))
        xpool = ctx.enter_context(tc.tile_pool(name="x", bufs=5))
        xnpool = ctx.enter_context(tc.tile_pool(name="xn", bufs=8))
        xtpool = ctx.enter_context(tc.tile_pool(name="xt", bufs=8))
        hpool = ctx.enter_context(tc.tile_pool(name="h", bufs=6))
        stats = ctx.enter_context(tc.tile_pool(name="stats", bufs=16))
        onepool = ctx.enter_context(tc.tile_pool(name="onehot", bufs=4))
        erow_pool = ctx.enter_context(tc.tile_pool(name="erow", bufs=2))
        dram = ctx.enter_context(tc.tile_pool(name="dram", bufs=2, space="DRAM"))
        ps_tp = ctx.enter_context(tc.tile_pool(name="ps_tp", bufs=2, space="PSUM"))
        ps_mm = ctx.enter_context(tc.tile_pool(name="ps_mm", bufs=3, space="PSUM"))
        ps_e = ctx.enter_context(tc.tile_pool(name="ps_e", bufs=2, space="PSUM"))
        ps_sg = ctx.enter_context(tc.tile_pool(name="ps_sg", bufs=1, space="PSUM"))

        # ---- constants -------------------------------------------------
        ident_f = consts.tile([P, P], F32)
        nc.gpsimd.memset(ident_f[:], 0.0)
        nc.gpsimd.affine_select(
            out=ident_f[:], in_=ident_f[:],
            compare_op=mybir.AluOpType.not_equal, fill=1.0,
            base=0, pattern=[[-1, P]], channel_multiplier=1,
        )
        ident = consts.tile([P, P], F32R)
        nc.vector.tensor_copy(out=ident[:], in_=ident_f[:])
        iota_t = consts.tile([P, N_MOL], F32)
        nc.gpsimd.iota(iota_t[:], pattern=[[1, N_MOL]], base=0,
                       channel_multiplier=0,
                       allow_small_or_imprecise_dtypes=True)
        bcol_t = consts.tile([P, NCHUNK], F32)
        nc.sync.dma_start(out=bcol_t[:], in_=bcol_d[:])
        cb_t = consts.tile([nrep, _src_tag()], F32)
        nc.sync.dma_start(out=cb_t[:], in_=cb_d[:])

        # weights: sync-DMA fp32, round to f32r on ScalarE (Copy shares the
        # Silu table set, and ACT is idle during warmup)
        wtmp_pool = ctx.enter_context(tc.tile_pool(name="wtmp", bufs=2))

        def wload(dram_t, kparts, n):
            tiles = []
            for kt in range(kparts):
                tmp = wtmp_pool.tile([P, n], F32, tag="wtmp")
                nc.sync.dma_start(out=tmp[:], in_=dram_t[kt * P:(kt + 1) * P, :])
                t = consts.tile([P, n], F32R, tag=f"w_{dram_t.name}_{kt}")
                nc.scalar.copy(out=t[:], in_=tmp[:])
                tiles.append(t)
            return tiles

        wps1_t = wload(wps1_d, F_PS // P, H)
        wps2_t = wload(wps2_d, H // P, H)
        wmp1_t = wload(wmp1_d, F_MP // P, H)
        wmp2_t = wload(wmp2_d, H // P, H)
        wps3_t = wload(wps3_d, 1, H // P)[0]
        wmp3_t = wload(wmp3_d, 1, H // P)[0]
        bps1_t = consts.tile([P, H // P], F32)
        nc.sync.dma_start(out=bps1_t[:], in_=bps1_d[:])
        bmp1_t = consts.tile([P, H // P], F32)
        nc.sync.dma_start(out=bmp1_t[:], in_=bmp1_d[:])

        # ---- main streams ---------------------------------------------
        def stream_sup(x_d, F, w1_t, b1_t, w2_t, w3_t, s, pe,
                       first_l3, last_l3, x3_pre=None):
            KT1 = F // P       # k-tiles of layer 1
            AT = SUP // P      # atom sub-tiles per supertile
            if True:
                a0 = s * SUP
                if x3_pre is not None:
                    x3 = x3_pre
                else:
                    x3 = xpool.tile([P, AT, F], F32, tag="x")
                    nc.sync.dma_start(
                        out=x3[:],
                        in_=x_d[a0:a0 + SUP, :].rearrange(
                            "(at p) f -> p at f", p=P),
                    )
                mv = stats.tile([P, AT, 2], F32, tag="mv")
                for at in range(AT):
                    st6 = stats.tile([P, 6], F32, tag="st6")
                    nc.vector.bn_stats(out=st6[:], in_=x3[:, at, :])
                    nc.vector.bn_aggr(out=mv[:, at, :], in_=st6[:])
                # rstd = rsqrt(var+eps) via bit-trick + 2 Newton steps (DVE)
                veps = stats.tile([P, AT], F32, tag="veps")
                nc.vector.tensor_scalar_add(veps[:], mv[:, :, 1], EPS)
                yi = stats.tile([P, AT], mybir.dt.int32, tag="yi")
                nc.vector.tensor_scalar(
                    out=yi[:], in0=veps[:].bitcast(mybir.dt.int32),
                    scalar1=1, scalar2=None,
                    op0=mybir.AluOpType.logical_shift_right)
                nc.vector.tensor_scalar(
                    out=yi[:], in0=yi[:],
                    scalar1=0x5F3759DF, scalar2=-1,
                    op0=mybir.AluOpType.subtract,
                    op1=mybir.AluOpType.mult)
                y = yi[:].bitcast(F32)
                tmp = stats.tile([P, AT], F32, tag="tmp")
                for _ in range(NEWTON_ITERS):
                    nc.vector.tensor_mul(tmp[:], veps[:], y)
                    nc.vector.tensor_mul(tmp[:], tmp[:], y)
                    nc.vector.tensor_scalar(
                        out=tmp[:], in0=tmp[:], scalar1=-0.5, scalar2=1.5,
                        op0=mybir.AluOpType.mult, op1=mybir.AluOpType.add)
                    nc.vector.tensor_mul(yi[:].bitcast(F32), y, tmp[:])
                rstd = yi[:].bitcast(F32)
                tsh = stats.tile([P, AT], F32, tag="tsh")
                nc.vector.tensor_scalar_mul(tsh[:], mv[:, :, 0], -1.0)
                nc.vector.tensor_mul(tsh[:], tsh[:], rstd)
                xn = []
                for at in range(AT):
                    xnt = xnpool.tile([P, F], F32R, tag="xn")
                    nc.gpsimd.tensor_scalar(
                        out=xnt[:], in0=x3[:, at, :],
                        scalar1=rstd[:, at:at + 1],
                        scalar2=tsh[:, at:at + 1],
                        op0=mybir.AluOpType.mult,
                        op1=mybir.AluOpType.add)
                    xn.append(xnt)
                xt = []
                for ft in range(KT1):
                    pxt = ps_tp.tile([P, SUP], F32R, space="PSUM", tag="pxt")
                    for at in range(AT):
                        nc.tensor.matmul(
                            out=pxt[:, at * P:(at + 1) * P],
                            lhsT=xn[at][:, ft * P:(ft + 1) * P],
                            rhs=ident[:],
                            is_transpose=True,
                            start=(at == 0), stop=(at == AT - 1),
                        )
                    xtt = xtpool.tile([P, SUP], F32R, tag="xt")
                    if F == F_PS:
                        nc.scalar.copy(out=xtt[:], in_=pxt[:])
                    else:
                        nc.vector.tensor_copy(out=xtt[:], in_=pxt[:])
                    xt.append(xtt)
                h1 = []
                for mt in range(H // P):
                    pg = ps_mm.tile([P, SUP], F32, space="PSUM", tag="pg")
                    for kt in range(KT1):
                        nc.tensor.matmul(
                            out=pg[:],
                            lhsT=w1_t[kt][:, mt * P:(mt + 1) * P],
                            rhs=xt[kt][:],
                            start=(kt == 0), stop=(kt == KT1 - 1),
                        )
                    h1t = hpool.tile([P, SUP], F32R, tag="h1")
                    nc.scalar.activation(out=h1t[:], in_=pg[:],
                                         func=ACTF.Silu,
                                         bias=b1_t[:, mt:mt + 1], scale=1.0)
                    h1.append(h1t)
                h2 = []
                for mt in range(H // P):
                    pg = ps_mm.tile([P, SUP], F32, space="PSUM", tag="pg")
                    for kt in range(H // P):
                        nc.tensor.matmul(
                            out=pg[:],
                            lhsT=w2_t[kt][:, mt * P:(mt + 1) * P],
                            rhs=h1[kt][:],
                            start=(kt == 0), stop=(kt == H // P - 1),
                        )
                    h2t = hpool.tile([P, SUP], F32R, tag="h2")
                    nc.scalar.activation(out=h2t[:], in_=pg[:],
                                         func=ACTF.Silu, scale=1.0)
                    h2.append(h2t)
                for kt in range(H // P):
                    nc.tensor.matmul(
                        out=pe[:],
                        lhsT=w3_t[:, kt:kt + 1],
                        rhs=h2[kt][:],
                        start=(first_l3 and kt == 0),
                        stop=(last_l3 and kt == H // P - 1),
                    )

        CPS = SUP // P  # segsum chunks per supertile
        seg_starts = _seg_starts()

        for _rep in range(nrep):
            ps_seg = ps_sg.tile([1, N_MOL], F32, space="PSUM", tag="pseg")
            for s in range(NSUP):
                # e_ps and e_mp L3s accumulate into one PSUM tile [1, SUP]
                pe = ps_e.tile([1, SUP], F32, space="PSUM", tag="pe")
                stream_sup(xps_d, F_PS, wps1_t, bps1_t, wps2_t, wps3_t,
                           s, pe, True, False)
                stream_sup(xmp_d, F_MP, wmp1_t, bmp1_t, wmp2_t, wmp3_t,
                           s, pe, False, True)
                e_sb = erow_pool.tile([1, SUP], F32, tag="e_sb")
                nc.vector.tensor_copy(out=e_sb[:], in_=pe[:])
                e_dram = dram.tile([SUP], F32, tag="e_dram")
                nc.sync.dma_start(out=e_dram[:], in_=e_sb[:])
                e_cols = erow_pool.tile([P, CPS], mybir.dt.float32r,
                                        tag="e_cols")
                nc.gpsimd.dma_start(
                    out=e_cols[:],
                    in_=e_dram[:].rearrange("(c p) -> p c", p=P),
                )
                for cc in range(CPS):
                    ch = s * CPS + cc
                    st, wid = seg_starts[ch]
                    oh = onepool.tile([P, N_MOL], mybir.dt.float32r,
                                      tag="oh")
                    nc.vector.tensor_scalar(
                        out=oh[:, :wid], in0=iota_t[:, :wid],
                        scalar1=bcol_t[:, ch:ch + 1],
                        scalar2=None, op0=mybir.AluOpType.is_equal,
                    )
                    nc.tensor.matmul(
                        out=ps_seg[0:1, st:st + wid],
                        lhsT=e_cols[:, cc:cc + 1], rhs=oh[:, :wid],
                        start=(ch == 0), stop=(ch == NCHUNK - 1),
                    )
            out_sb = erow_pool.tile([1, N_MOL], F32, tag="out_sb")
            nc.vector.tensor_copy(out=out_sb[:], in_=ps_seg[:])
            nc.sync.dma_start(out=out_d[:], in_=out_sb[:])

    nc.compile()
    _CACHE[("nc", nrep)] = nc
    return nc


def _shard_inputs(x_ps, x_mp, batch, gamma_ps, beta_ps, gamma_mp, beta_mp,
                  W_ps1, W_ps2, W_ps3, W_mp1, W_mp2, W_mp3, nrep=1):
    f32 = np.float32
    cachebust = np.zeros((nrep, _src_tag()), f32)
    x_ps = np.asarray(x_ps, dtype=f32)
    x_mp = np.asarray(x_mp, dtype=f32)
    batch = np.asarray(batch).astype(np.int64)
    sw = _seg_starts()
    starts = np.array([s for s, _ in sw])
    widths = np.array([w for _, w in sw])
    blk = N_ATOMS // N_SPECIES
    in_maps = []
    for c in range(N_CORES):
        s = c // (N_CORES // N_SPECIES)
        h = c % (N_CORES // N_SPECIES)
        bb = batch[s * blk:(s + 1) * blk]
        perm = np.argsort(bb, kind="stable")[h * APC:(h + 1) * APC]
        gidx = s * blk + perm
        bs = bb[perm]
        # local molecule coords: shift by -256*h +64 guard, then per-chunk
        # window start subtraction (window membership asserted below)
        shifted = bs - (N_MOL // 2) * h + 64
        bc = shifted.reshape(NCHUNK, P) - starts[:, None]
        assert (bc >= 0).all() and (bc < widths[:, None]).all(), \
            "segment window overflow - pathological batch distribution"
        w1p = (np.asarray(gamma_ps, f32)[:, None] * np.asarray(W_ps1[s], f32))
        b1p = (np.asarray(beta_ps, f32) @ np.asarray(W_ps1[s], f32))
        w1m = (np.asarray(gamma_mp, f32)[:, None] * np.asarray(W_mp1[s], f32))
        b1m = (np.asarray(beta_mp, f32) @ np.asarray(W_mp1[s], f32))
        in_maps.append({
            "cachebust": cachebust,
            "xps": np.ascontiguousarray(x_ps[gidx]),
            "xmp": np.ascontiguousarray(x_mp[gidx]),
            "bcol": np.ascontiguousarray(bc.T.astype(f32)),
            "wps1": np.ascontiguousarray(w1p.astype(f32)),
            "bps1": np.ascontiguousarray(b1p.astype(f32).reshape(H // P, P).T),
            "wps2": np.ascontiguousarray(np.asarray(W_ps2[s], dtype=f32)),
            "wps3": np.ascontiguousarray(
                np.asarray(W_ps3[s], dtype=f32)[:, 0].reshape(H // P, P).T),
            "wmp1": np.ascontiguousarray(w1m.astype(f32)),
            "bmp1": np.ascontiguousarray(b1m.astype(f32).reshape(H // P, P).T),
            "wmp2": np.ascontiguousarray(np.asarray(W_mp2[s], dtype=f32)),
            "wmp3": np.ascontiguousarray(
                np.asarray(W_mp3[s], dtype=f32)[:, 0].reshape(H // P, P).T),
        })
    return in_maps


def _gather_output(partials):
    """Sum per-core partial energies, undoing each core's local molecule
    coordinate shift (local j corresponds to global m = j + 256*h - 64)."""
    full = np.zeros(N_MOL, dtype=np.float64)
    for c, part in enumerate(partials):
        h = c % (N_CORES // N_SPECIES)
        off = (N_MOL // 2) * h - 64
        j = np.arange(N_MOL)
        m = j + off
        valid = (m >= 0) & (m < N_MOL)
        np.add.at(full, m[valid], part.astype(np.float64)[valid])
    return full.astype(np.float32)


def kernel(x_ps, x_mp, batch, gamma_ps, beta_ps, gamma_mp, beta_mp,
           W_ps1, W_ps2, W_ps3, W_mp1, W_mp2, W_mp3, _want_results=False):
    from concourse.bass_utils import run_bass_kernel_spmd

    nc = _build()
    in_maps = _shard_inputs(
        x_ps, x_mp, batch, gamma_ps, beta_ps, gamma_mp, beta_mp,
        W_ps1, W_ps2, W_ps3, W_mp1, W_mp2, W_mp3)
    res = run_bass_kernel_spmd(nc, in_maps, list(range(N_CORES)))
    partials = [res.results[c]["out"] for c in range(N_CORES)]
    out = _gather_output(partials).reshape(N_MOL, 1)
    if _want_results:
        return out, res
    return out


# revision 50
# speedup vs baseline: 2.2903x; 1.3556x over previous
"""Trainium2 Bass kernel for nn_BPPSLodeModel (moe_routing).

Model (per reference):
  f_ps = LayerNorm(x_ps) * gamma_ps + beta_ps        # [N, 512]
  f_mp = LayerNorm(x_mp) * gamma_mp + beta_mp        # [N, 256]
  e_ps = species_mlp(f_ps, W_ps1/2/3)                # [N, 1], 4 contiguous species blocks
  e_mp = species_mlp(f_mp, W_mp1/2/3)                # [N, 1]
  out  = segment_sum(e_ps + e_mp, batch, 512)        # [512, 1]

Sharding: data-parallel over atoms. 65536 atoms / 8 cores = 8192 atoms per
core; species blocks are 16384 atoms, so core c holds species c//2 only and
receives just that species' weights. gamma/beta are folded into W1/b1 on the
host (exact: LN(x)*g+b @ W1 == LN(x) @ (g*W1) + b@W1). Each core computes a
partial [512] per-molecule energy vector over its atoms; the host sums the 8
partials (the gather step of the data-parallel reduction).

On-core dataflow (all matmuls in float32r, fp32 PSUM accumulate):
  DMA x (atom-major) -> bn_stats/bn_aggr (DVE) -> rstd,t (small ops)
  -> normalize on ScalarE (x*rstd + t, rounds to f32r)
  -> PE transpose to feature-major X^T -> DVE copy PSUM->SBUF
  -> L1/L2 matmuls (weights stationary) + SiLU(+bias) on ScalarE
  -> L3 (M=1) -> per-atom energy row e[1, atoms]
  -> e reshaped to [128, chunk] columns via DRAM bounce
  -> segment-sum: one-hot(batch)==iota matmuls accumulating into PSUM [1,512]
"""

import numpy as np

N_ATOMS = 65536
N_CORES = 8
APC = N_ATOMS // N_CORES        # atoms per core = 8192
N_SPECIES = 4
N_MOL = 512
F_PS = 512
F_MP = 256
H = 256
EPS = 1e-5
SUP = 512                       # atoms per supertile
NSUP = APC // SUP               # 16
P = 128
NCHUNK = APC // P               # 64 segment-sum chunks
NEWTON_ITERS = 2                # rsqrt Newton refinements (rel err ~5e-6)
SEG_WIN = 256                   # molecule window per chunk (atoms host-sorted)


def _seg_starts():
    # Atoms are host-sorted by molecule and shifted to local coords with a
    # +64 guard: chunk ch's atoms sit near local molecule 4*ch + 64, with
    # statistical drift well under +-40.  A 256-wide window is bulletproof.
    # Chunk 0 uses the full 512 so its start=True matmul claims and writes
    # the whole PSUM bank (idempotent across executions, sim-exact).
    sw = [(min(max(4 * ch - 60, 0), N_MOL - SEG_WIN), SEG_WIN)
          for ch in range(NCHUNK)]
    sw[0] = (0, N_MOL)
    return sw

_CACHE = {}


def _src_tag():
    import hashlib
    try:
        with open(__file__, "rb") as f:
            return int(hashlib.md5(f.read()).hexdigest()[:4], 16) % 509 + 2
    except OSError:
        return 1


def _build(nrep=1):
    if ("nc", nrep) in _CACHE:
        return _CACHE[("nc", nrep)]

    import concourse.bacc as bacc
    import concourse.tile as tile
    from concourse import mybir

    F32 = mybir.dt.float32
    F32R = mybir.dt.float32r
    ACTF = mybir.ActivationFunctionType

    nc = bacc.Bacc("TRN2", target_bir_lowering=False, debug=False,
                   num_devices=N_CORES)

    xps_d = nc.dram_tensor("xps", [APC, F_PS], F32, kind="ExternalInput")
    xmp_d = nc.dram_tensor("xmp", [APC, F_MP], F32, kind="ExternalInput")
    bcol_d = nc.dram_tensor("bcol", [P, NCHUNK], F32, kind="ExternalInput")
    wps1_d = nc.dram_tensor("wps1", [F_PS, H], F32, kind="ExternalInput")
    bps1_d = nc.dram_tensor("bps1", [P, H // P], F32, kind="ExternalInput")
    wps2_d = nc.dram_tensor("wps2", [H, H], F32, kind="ExternalInput")
    wps3_d = nc.dram_tensor("wps3", [P, H // P], F32, kind="ExternalInput")
    wmp1_d = nc.dram_tensor("wmp1", [F_MP, H], F32, kind="ExternalInput")
    bmp1_d = nc.dram_tensor("bmp1", [P, H // P], F32, kind="ExternalInput")
    wmp2_d = nc.dram_tensor("wmp2", [H, H], F32, kind="ExternalInput")
    wmp3_d = nc.dram_tensor("wmp3", [P, H // P], F32, kind="ExternalInput")
    out_d = nc.dram_tensor("out", [N_MOL], F32, kind="ExternalOutput")
    # NEFF/jit caches key on the I/O signature, not the kernel body: encode
    # (source hash, nrep) into a dummy input's shape so edits recompile.
    cb_d = nc.dram_tensor("cachebust", [nrep, _src_tag()], F32,
                          kind="ExternalInput")

    from contextlib import ExitStack
    with tile.TileContext(nc) as tc, ExitStack() as ctx:
        consts = ctx.enter_context(tc.tile_pool(name="consts", bufs=1))
        xpool = ctx.enter_context(tc.tile_pool(name="x", bufs=5))
        xnpool = ctx.enter_context(tc.tile_pool(name="xn", bufs=8))
        xtpool = ctx.enter_context(tc.tile_pool(name="xt", bufs=8))
        hpool = ctx.enter_context(tc.tile_pool(name="h", bufs=6))
        stats = ctx.enter_context(tc.tile_pool(name="stats", bufs=16))
        onepool = ctx.enter_context(tc.tile_pool(name="onehot", bufs=4))
        erow_pool = ctx.enter_context(tc.tile_pool(name="erow", bufs=2))
        dram = ctx.enter_context(tc.tile_pool(name="dram", bufs=2, space="DRAM"))
        ps_tp = ctx.enter_context(tc.tile_pool(name="ps_tp", bufs=2, space="PSUM"))
        ps_mm = ctx.enter_context(tc.tile_pool(name="ps_mm", bufs=3, space="PSUM"))
        ps_e = ctx.enter_context(tc.tile_pool(name="ps_e", bufs=2, space="PSUM"))
        ps_sg = ctx.enter_context(tc.tile_pool(name="ps_sg", bufs=1, space="PSUM"))

        # ---- constants -------------------------------------------------
        ident_f = consts.tile([P, P], F32)
        nc.gpsimd.memset(ident_f[:], 0.0)
        nc.gpsimd.affine_select(
            out=ident_f[:], in_=ident_f[:],
            compare_op=mybir.AluOpType.not_equal, fill=1.0,
            base=0, pattern=[[-1, P]], channel_multiplier=1,
        )
        ident = consts.tile([P, P], F32R)
        nc.vector.tensor_copy(out=ident[:], in_=ident_f[:])
        iota_t = consts.tile([P, N_MOL], F32)
        nc.gpsimd.iota(iota_t[:], pattern=[[1, N_MOL]], base=0,
                       channel_multiplier=0,
                       allow_small_or_imprecise_dtypes=True)
        bcol_t = consts.tile([P, NCHUNK], F32)
        nc.sync.dma_start(out=bcol_t[:], in_=bcol_d[:])
        cb_t = consts.tile([nrep, _src_tag()], F32)
        nc.sync.dma_start(out=cb_t[:], in_=cb_d[:])

        # weights: sync-DMA fp32, round to f32r on ScalarE (Copy shares the
        # Silu table set, and ACT is idle during warmup)
        wtmp_pool = ctx.enter_context(tc.tile_pool(name="wtmp", bufs=2))

        def wload(dram_t, kparts, n):
            tiles = []
            for kt in range(kparts):
                tmp = wtmp_pool.tile([P, n], F32, tag="wtmp")
                nc.sync.dma_start(out=tmp[:], in_=dram_t[kt * P:(kt + 1) * P, :])
                t = consts.tile([P, n], F32R, tag=f"w_{dram_t.name}_{kt}")
                nc.scalar.copy(out=t[:], in_=tmp[:])
                tiles.append(t)
            return tiles

        wps1_t = wload(wps1_d, F_PS // P, H)
        wps2_t = wload(wps2_d, H // P, H)
        wmp1_t = wload(wmp1_d, F_MP // P, H)
        wmp2_t = wload(wmp2_d, H // P, H)
        wps3_t = wload(wps3_d, 1, H // P)[0]
        wmp3_t = wload(wmp3_d, 1, H // P)[0]
        bps1_t = consts.tile([P, H // P], F32)
        nc.sync.dma_start(out=bps1_t[:], in_=bps1_d[:])
        bmp1_t = consts.tile([P, H // P], F32)
        nc.sync.dma_start(out=bmp1_t[:], in_=bmp1_d[:])

        # ---- main streams ---------------------------------------------
        def stream_sup(x_d, F, w1_t, b1_t, w2_t, w3_t, s, pe,
                       first_l3, last_l3, x3_pre=None):
            KT1 = F // P       # k-tiles of layer 1
            AT = SUP // P      # atom sub-tiles per supertile
            if True:
                a0 = s * SUP
                if x3_pre is not None:
                    x3 = x3_pre
                else:
                    x3 = xpool.tile([P, AT, F], F32, tag="x")
                    nc.sync.dma_start(
                        out=x3[:],
                        in_=x_d[a0:a0 + SUP, :].rearrange(
                            "(at p) f -> p at f", p=P),
                    )
                mv = stats.tile([P, AT, 2], F32, tag="mv")
                for at in range(AT):
                    st6 = stats.tile([P, 6], F32, tag="st6")
                    nc.vector.bn_stats(out=st6[:], in_=x3[:, at, :])
                    nc.vector.bn_aggr(out=mv[:, at, :], in_=st6[:])
                # rstd = rsqrt(var+eps) via bit-trick + 2 Newton steps (DVE)
                veps = stats.tile([P, AT], F32, tag="veps")
                nc.vector.tensor_scalar_add(veps[:], mv[:, :, 1], EPS)
                yi = stats.tile([P, AT], mybir.dt.int32, tag="yi")
                nc.vector.tensor_scalar(
                    out=yi[:], in0=veps[:].bitcast(mybir.dt.int32),
                    scalar1=1, scalar2=None,
                    op0=mybir.AluOpType.logical_shift_right)
                nc.vector.tensor_scalar(
                    out=yi[:], in0=yi[:],
                    scalar1=0x5F3759DF, scalar2=-1,
                    op0=mybir.AluOpType.subtract,
                    op1=mybir.AluOpType.mult)
                y = yi[:].bitcast(F32)
                tmp = stats.tile([P, AT], F32, tag="tmp")
                for _ in range(NEWTON_ITERS):
                    nc.vector.tensor_mul(tmp[:], veps[:], y)
                    nc.vector.tensor_mul(tmp[:], tmp[:], y)
                    nc.vector.tensor_scalar(
                        out=tmp[:], in0=tmp[:], scalar1=-0.5, scalar2=1.5,
                        op0=mybir.AluOpType.mult, op1=mybir.AluOpType.add)
                    nc.vector.tensor_mul(yi[:].bitcast(F32), y, tmp[:])
                rstd = yi[:].bitcast(F32)
                tsh = stats.tile([P, AT], F32, tag="tsh")
                nc.vector.tensor_scalar_mul(tsh[:], mv[:, :, 0], -1.0)
                nc.vector.tensor_mul(tsh[:], tsh[:], rstd)
                xn = []
                for at in range(AT):
                    xnt = xnpool.tile([P, F], F32R, tag="xn")
                    nc.gpsimd.tensor_scalar(
                        out=xnt[:], in0=x3[:, at, :],
                        scalar1=rstd[:, at:at + 1],
                        scalar2=tsh[:, at:at + 1],
                        op0=mybir.AluOpType.mult,
                        op1=mybir.AluOpType.add)
                    xn.append(xnt)
                xt = []
                for ft in range(KT1):
                    pxt = ps_tp.tile([P, SUP], F32R, space="PSUM", tag="pxt")
                    for at in range(AT):
                        nc.tensor.matmul(
                            out=pxt[:, at * P:(at + 1) * P],
                            lhsT=xn[at][:, ft * P:(ft + 1) * P],
                            rhs=ident[:],
                            is_transpose=True,
                            start=(at == 0), stop=(at == AT - 1),
                        )
                    xtt = xtpool.tile([P, SUP], F32R, tag="xt")
                    if F == F_PS:
                        nc.scalar.copy(out=xtt[:], in_=pxt[:])
                    else:
                        nc.vector.tensor_copy(out=xtt[:], in_=pxt[:])
                    xt.append(xtt)
                h1 = []
                for mt in range(H // P):
                    pg = ps_mm.tile([P, SUP], F32, space="PSUM", tag="pg")
                    for kt in range(KT1):
                        nc.tensor.matmul(
                            out=pg[:],
                            lhsT=w1_t[kt][:, mt * P:(mt + 1) * P],
                            rhs=xt[kt][:],
                            start=(kt == 0), stop=(kt == KT1 - 1),
                        )
                    h1t = hpool.tile([P, SUP], F32R, tag="h1")
                    nc.scalar.activation(out=h1t[:], in_=pg[:],
                                         func=ACTF.Silu,
                                         bias=b1_t[:, mt:mt + 1], scale=1.0)
                    h1.append(h1t)
                h2 = []
                for mt in range(H // P):
                    pg = ps_mm.tile([P, SUP], F32, space="PSUM", tag="pg")
                    for kt in range(H // P):
                        nc.tensor.matmul(
                            out=pg[:],
                            lhsT=w2_t[kt][:, mt * P:(mt + 1) * P],
                            rhs=h1[kt][:],
                            start=(kt == 0), stop=(kt == H // P - 1),
                        )
                    h2t = hpool.tile([P, SUP], F32R, tag="h2")
                    nc.scalar.activation(out=h2t[:], in_=pg[:],
                                         func=ACTF.Silu, scale=1.0)
                    h2.append(h2t)
                for kt in range(H // P):
                    nc.tensor.matmul(
                        out=pe[:],
                        lhsT=w3_t[:, kt:kt + 1],
                        rhs=h2[kt][:],
                        start=(first_l3 and kt == 0),
                        stop=(last_l3 and kt == H // P - 1),
                    )

        CPS = SUP // P  # segsum chunks per supertile
        seg_starts = _seg_starts()

        for _rep in range(nrep):
            ps_seg = ps_sg.tile([1, N_MOL], F32, space="PSUM", tag="pseg")
            for s in range(NSUP):
                # e_ps and e_mp L3s accumulate into one PSUM tile [1, SUP]
                pe = ps_e.tile([1, SUP], F32, space="PSUM", tag="pe")
                stream_sup(xps_d, F_PS, wps1_t, bps1_t, wps2_t, wps3_t,
                           s, pe, True, False)
                stream_sup(xmp_d, F_MP, wmp1_t, bmp1_t, wmp2_t, wmp3_t,
                           s, pe, False, True)
                e_sb = erow_pool.tile([1, SUP], F32, tag="e_sb")
                nc.vector.tensor_copy(out=e_sb[:], in_=pe[:])
                e_dram = dram.tile([SUP], F32, tag="e_dram")
                nc.sync.dma_start(out=e_dram[:], in_=e_sb[:])
                e_cols = erow_pool.tile([P, CPS], mybir.dt.float32r,
                                        tag="e_cols")
                nc.gpsimd.dma_start(
                    out=e_cols[:],
                    in_=e_dram[:].rearrange("(c p) -> p c", p=P),
                )
                for cc in range(CPS):
                    ch = s * CPS + cc
                    st, wid = seg_starts[ch]
                    oh = onepool.tile([P, N_MOL], mybir.dt.float32r,
                                      tag="oh")
                    nc.vector.tensor_scalar(
                        out=oh[:, :wid], in0=iota_t[:, :wid],
                        scalar1=bcol_t[:, ch:ch + 1],
                        scalar2=None, op0=mybir.AluOpType.is_equal,
                    )
                    nc.tensor.matmul(
                        out=ps_seg[0:1, st:st + wid],
                        lhsT=e_cols[:, cc:cc + 1], rhs=oh[:, :wid],
                        start=(ch == 0), stop=(ch == NCHUNK - 1),
                    )
            out_sb = erow_pool.tile([1, N_MOL], F32, tag="out_sb")
            nc.vector.tensor_copy(out=out_sb[:], in_=ps_seg[:])
            nc.sync.dma_start(out=out_d[:], in_=out_sb[:])

    nc.compile()
    _CACHE[("nc", nrep)] = nc
    return nc


def _shard_inputs(x_ps, x_mp, batch, gamma_ps, beta_ps, gamma_mp, beta_mp,
                  W_ps1, W_ps2, W_ps3, W_mp1, W_mp2, W_mp3, nrep=1):
    f32 = np.float32
    cachebust = np.zeros((nrep, _src_tag()), f32)
    x_ps = np.asarray(x_ps, dtype=f32)
    x_mp = np.asarray(x_mp, dtype=f32)
    batch = np.asarray(batch).astype(np.int64)
    sw = _seg_starts()
    starts = np.array([s for s, _ in sw])
    widths = np.array([w for _, w in sw])
    blk = N_ATOMS // N_SPECIES
    in_maps = []
    for c in range(N_CORES):
        s = c // (N_CORES // N_SPECIES)
        h = c % (N_CORES // N_SPECIES)
        bb = batch[s * blk:(s + 1) * blk]
        perm = np.argsort(bb, kind="stable")[h * APC:(h + 1) * APC]
        gidx = s * blk + perm
        bs = bb[perm]
        # local molecule coords: shift by -256*h +64 guard, then per-chunk
        # window start subtraction (window membership asserted below)
        shifted = bs - (N_MOL // 2) * h + 64
        bc = shifted.reshape(NCHUNK, P) - starts[:, None]
        assert (bc >= 0).all() and (bc < widths[:, None]).all(), \
            "segment window overflow - pathological batch distribution"
        w1p = (np.asarray(gamma_ps, f32)[:, None] * np.asarray(W_ps1[s], f32))
        b1p = (np.asarray(beta_ps, f32) @ np.asarray(W_ps1[s], f32))
        w1m = (np.asarray(gamma_mp, f32)[:, None] * np.asarray(W_mp1[s], f32))
        b1m = (np.asarray(beta_mp, f32) @ np.asarray(W_mp1[s], f32))
        in_maps.append({
            "cachebust": cachebust,
            "xps": np.ascontiguousarray(x_ps[gidx]),
            "xmp": np.ascontiguousarray(x_mp[gidx]),
            "bcol": np.ascontiguousarray(bc.T.astype(f32)),
            "wps1": np.ascontiguousarray(w1p.astype(f32)),
            "bps1": np.ascontiguousarray(b1p.astype(f32).reshape(H // P, P).T),
            "wps2": np.ascontiguousarray(np.asarray(W_ps2[s], dtype=f32)),
            "wps3": np.ascontiguousarray(
                np.asarray(W_ps3[s], dtype=f32)[:, 0].reshape(H // P, P).T),
            "wmp1": np.ascontiguousarray(w1m.astype(f32)),
            "bmp1": np.ascontiguousarray(b1m.astype(f32).reshape(H // P, P).T),
            "wmp2": np.ascontiguousarray(np.asarray(W_mp2[s], dtype=f32)),
            "wmp3": np.ascontiguousarray(
                np.asarray(W_mp3[s], dtype=f32)[:, 0].reshape(H // P, P).T),
        })
    return in_maps


def _gather_output(partials):
    """Sum per-core partial energies, undoing each core's local molecule
    coordinate shift (local j corresponds to global m = j + 256*h - 64)."""
    full = np.zeros(N_MOL, dtype=np.float64)
    for c, part in enumerate(partials):
        h = c % (N_CORES // N_SPECIES)
        off = (N_MOL // 2) * h - 64
        j = np.arange(N_MOL)
        m = j + off
        valid = (m >= 0) & (m < N_MOL)
        np.add.at(full, m[valid], part.astype(np.float64)[valid])
    return full.astype(np.float32)


def kernel(x_ps, x_mp, batch, gamma_ps, beta_ps, gamma_mp, beta_mp,
           W_ps1, W_ps2, W_ps3, W_mp1, W_mp2, W_mp3, _want_results=False):
    from concourse.bass_utils import run_bass_kernel_spmd

    nc = _build()
    in_maps = _shard_inputs(
        x_ps, x_mp, batch, gamma_ps, beta_ps, gamma_mp, beta_mp,
        W_ps1, W_ps2, W_ps3, W_mp1, W_mp2, W_mp3)
    res = run_bass_kernel_spmd(nc, in_maps, list(range(N_CORES)))
    partials = [res.results[c]["out"] for c in range(N_CORES)]
    out = _gather_output(partials).reshape(N_MOL, 1)
    if _want_results:
        return out, res
    return out


# revision 54
# speedup vs baseline: 17.9528x; 7.8387x over previous
"""Trainium2 Bass kernel for nn_BPPSLodeModel (moe_routing).

Model (per reference):
  f_ps = LayerNorm(x_ps) * gamma_ps + beta_ps        # [N, 512]
  f_mp = LayerNorm(x_mp) * gamma_mp + beta_mp        # [N, 256]
  e_ps = species_mlp(f_ps, W_ps1/2/3)                # [N, 1], 4 contiguous species blocks
  e_mp = species_mlp(f_mp, W_mp1/2/3)                # [N, 1]
  out  = segment_sum(e_ps + e_mp, batch, 512)        # [512, 1]

Sharding: data-parallel over atoms. 65536 atoms / 8 cores = 8192 atoms per
core; species blocks are 16384 atoms, so core c holds species c//2 only and
receives just that species' weights. gamma/beta are folded into W1/b1 on the
host (exact: LN(x)*g+b @ W1 == LN(x) @ (g*W1) + b@W1). Each core computes a
partial [512] per-molecule energy vector over its atoms; the host sums the 8
partials (the gather step of the data-parallel reduction).

On-core dataflow (all matmuls in float32r, fp32 PSUM accumulate):
  DMA x (atom-major) -> bn_stats/bn_aggr (DVE) -> rstd,t (small ops)
  -> normalize on ScalarE (x*rstd + t, rounds to f32r)
  -> PE transpose to feature-major X^T -> DVE copy PSUM->SBUF
  -> L1/L2 matmuls (weights stationary) + SiLU(+bias) on ScalarE
  -> L3 (M=1) -> per-atom energy row e[1, atoms]
  -> e reshaped to [128, chunk] columns via DRAM bounce
  -> segment-sum: one-hot(batch)==iota matmuls accumulating into PSUM [1,512]
"""

import numpy as np

N_ATOMS = 65536
N_CORES = 8
APC = N_ATOMS // N_CORES        # atoms per core = 8192
N_SPECIES = 4
N_MOL = 512
F_PS = 512
F_MP = 256
H = 256
EPS = 1e-5
SUP = 512                       # atoms per supertile
NSUP = APC // SUP               # 16
P = 128
NCHUNK = APC // P               # 64 segment-sum chunks
NEWTON_ITERS = 2                # rsqrt Newton refinements (rel err ~5e-6)
SEG_WIN = 256                   # molecule window per chunk (atoms host-sorted)


def _seg_starts():
    # Atoms are host-sorted by molecule and shifted to local coords with a
    # +64 guard: chunk ch's atoms sit near local molecule 4*ch + 64, with
    # statistical drift well under +-40.  A 256-wide window is bulletproof.
    # Chunk 0 uses the full 512 so its start=True matmul claims and writes
    # the whole PSUM bank (idempotent across executions, sim-exact).
    sw = [(min(max(4 * ch - 60, 0), N_MOL - SEG_WIN), SEG_WIN)
          for ch in range(NCHUNK)]
    sw[0] = (0, N_MOL)
    return sw

_CACHE = {}


def _src_tag():
    import hashlib
    try:
        with open(__file__, "rb") as f:
            return int(hashlib.md5(f.read()).hexdigest()[:4], 16) % 509 + 2
    except OSError:
        return 1


def _build(nrep=1):
    if ("nc", nrep) in _CACHE:
        return _CACHE[("nc", nrep)]

    import concourse.bacc as bacc
    import concourse.tile as tile
    from concourse import mybir

    F32 = mybir.dt.float32
    F32R = mybir.dt.float32r
    ACTF = mybir.ActivationFunctionType

    nc = bacc.Bacc("TRN2", target_bir_lowering=False, debug=False,
                   num_devices=N_CORES)

    xps_d = nc.dram_tensor("xps", [APC, F_PS], F32, kind="ExternalInput")
    xmp_d = nc.dram_tensor("xmp", [APC, F_MP], F32, kind="ExternalInput")
    bcol_d = nc.dram_tensor("bcol", [P, NCHUNK], F32, kind="ExternalInput")
    wps1_d = nc.dram_tensor("wps1", [F_PS, H], F32, kind="ExternalInput")
    bps1_d = nc.dram_tensor("bps1", [P, H // P], F32, kind="ExternalInput")
    wps2_d = nc.dram_tensor("wps2", [H, H], F32, kind="ExternalInput")
    wps3_d = nc.dram_tensor("wps3", [P, H // P], F32, kind="ExternalInput")
    wmp1_d = nc.dram_tensor("wmp1", [F_MP, H], F32, kind="ExternalInput")
    bmp1_d = nc.dram_tensor("bmp1", [P, H // P], F32, kind="ExternalInput")
    wmp2_d = nc.dram_tensor("wmp2", [H, H], F32, kind="ExternalInput")
    wmp3_d = nc.dram_tensor("wmp3", [P, H // P], F32, kind="ExternalInput")
    out_d = nc.dram_tensor("out", [N_MOL], F32, kind="ExternalOutput")
    # NEFF/jit caches key on the I/O signature, not the kernel body: encode
    # (source hash, nrep) into a dummy input's shape so edits recompile.
    cb_d = nc.dram_tensor("cachebust", [nrep, _src_tag()], F32,
                          kind="ExternalInput")

    from contextlib import ExitStack
    with tile.TileContext(nc) as tc, ExitStack() as ctx:
        consts = ctx.enter_context(tc.tile_pool(name="consts", bufs=1))
        xpool = ctx.enter_context(tc.tile_pool(name="x", bufs=5))
        xnpool = ctx.enter_context(tc.tile_pool(name="xn", bufs=8))
        xtpool = ctx.enter_context(tc.tile_pool(name="xt", bufs=8))
        hpool = ctx.enter_context(tc.tile_pool(name="h", bufs=6))
        stats = ctx.enter_context(tc.tile_pool(name="stats", bufs=16))
        onepool = ctx.enter_context(tc.tile_pool(name="onehot", bufs=4))
        erow_pool = ctx.enter_context(tc.tile_pool(name="erow", bufs=2))
        dram = ctx.enter_context(tc.tile_pool(name="dram", bufs=2, space="DRAM"))
        ps_tp = ctx.enter_context(tc.tile_pool(name="ps_tp", bufs=2, space="PSUM"))
        ps_mm = ctx.enter_context(tc.tile_pool(name="ps_mm", bufs=3, space="PSUM"))
        ps_e = ctx.enter_context(tc.tile_pool(name="ps_e", bufs=2, space="PSUM"))
        ps_sg = ctx.enter_context(tc.tile_pool(name="ps_sg", bufs=1, space="PSUM"))

        # ---- constants -------------------------------------------------
        ident_f = consts.tile([P, P], F32)
        nc.gpsimd.memset(ident_f[:], 0.0)
        nc.gpsimd.affine_select(
            out=ident_f[:], in_=ident_f[:],
            compare_op=mybir.AluOpType.not_equal, fill=1.0,
            base=0, pattern=[[-1, P]], channel_multiplier=1,
        )
        ident = consts.tile([P, P], F32R)
        nc.vector.tensor_copy(out=ident[:], in_=ident_f[:])
        iota_t = consts.tile([P, N_MOL], F32)
        nc.gpsimd.iota(iota_t[:], pattern=[[1, N_MOL]], base=0,
                       channel_multiplier=0,
                       allow_small_or_imprecise_dtypes=True)
        bcol_t = consts.tile([P, NCHUNK], F32)
        nc.sync.dma_start(out=bcol_t[:], in_=bcol_d[:])
        cb_t = consts.tile([nrep, _src_tag()], F32)
        nc.sync.dma_start(out=cb_t[:], in_=cb_d[:])

        # weights: sync-DMA fp32, round to f32r on ScalarE (Copy shares the
        # Silu table set, and ACT is idle during warmup)
        wtmp_pool = ctx.enter_context(tc.tile_pool(name="wtmp", bufs=2))

        def wload(dram_t, kparts, n):
            tiles = []
            for kt in range(kparts):
                tmp = wtmp_pool.tile([P, n], F32, tag="wtmp")
                nc.sync.dma_start(out=tmp[:], in_=dram_t[kt * P:(kt + 1) * P, :])
                t = consts.tile([P, n], F32R, tag=f"w_{dram_t.name}_{kt}")
                nc.scalar.copy(out=t[:], in_=tmp[:])
                tiles.append(t)
            return tiles

        wps1_t = wload(wps1_d, F_PS // P, H)
        wps2_t = wload(wps2_d, H // P, H)
        wmp1_t = wload(wmp1_d, F_MP // P, H)
        wmp2_t = wload(wmp2_d, H // P, H)
        wps3_t = wload(wps3_d, 1, H // P)[0]
        wmp3_t = wload(wmp3_d, 1, H // P)[0]
        bps1_t = consts.tile([P, H // P], F32)
        nc.sync.dma_start(out=bps1_t[:], in_=bps1_d[:])
        bmp1_t = consts.tile([P, H // P], F32)
        nc.sync.dma_start(out=bmp1_t[:], in_=bmp1_d[:])

        # ---- main streams ---------------------------------------------
        def stream_sup(x_d, F, w1_t, b1_t, w2_t, w3_t, s, pe,
                       first_l3, last_l3, x3_pre=None):
            KT1 = F // P       # k-tiles of layer 1
            AT = SUP // P      # atom sub-tiles per supertile
            if True:
                a0 = s * SUP
                if x3_pre is not None:
                    x3 = x3_pre
                else:
                    x3 = xpool.tile([P, AT, F], F32, tag="x")
                    nc.sync.dma_start(
                        out=x3[:],
                        in_=x_d[a0:a0 + SUP, :].rearrange(
                            "(at p) f -> p at f", p=P),
                    )
                mv = stats.tile([P, AT, 2], F32, tag="mv")
                for at in range(AT):
                    st6 = stats.tile([P, 6], F32, tag="st6")
                    nc.vector.bn_stats(out=st6[:], in_=x3[:, at, :])
                    nc.vector.bn_aggr(out=mv[:, at, :], in_=st6[:])
                # rstd = rsqrt(var+eps) via bit-trick + 2 Newton steps (DVE)
                veps = stats.tile([P, AT], F32, tag="veps")
                nc.vector.tensor_scalar_add(veps[:], mv[:, :, 1], EPS)
                yi = stats.tile([P, AT], mybir.dt.int32, tag="yi")
                nc.vector.tensor_scalar(
                    out=yi[:], in0=veps[:].bitcast(mybir.dt.int32),
                    scalar1=1, scalar2=None,
                    op0=mybir.AluOpType.logical_shift_right)
                nc.vector.tensor_scalar(
                    out=yi[:], in0=yi[:],
                    scalar1=0x5F3759DF, scalar2=-1,
                    op0=mybir.AluOpType.subtract,
                    op1=mybir.AluOpType.mult)
                y = yi[:].bitcast(F32)
                tmp = stats.tile([P, AT], F32, tag="tmp")
                for _ in range(NEWTON_ITERS):
                    nc.vector.tensor_mul(tmp[:], veps[:], y)
                    nc.vector.tensor_mul(tmp[:], tmp[:], y)
                    nc.vector.tensor_scalar(
                        out=tmp[:], in0=tmp[:], scalar1=-0.5, scalar2=1.5,
                        op0=mybir.AluOpType.mult, op1=mybir.AluOpType.add)
                    nc.vector.tensor_mul(yi[:].bitcast(F32), y, tmp[:])
                rstd = yi[:].bitcast(F32)
                tsh = stats.tile([P, AT], F32, tag="tsh")
                nc.vector.tensor_scalar_mul(tsh[:], mv[:, :, 0], -1.0)
                nc.vector.tensor_mul(tsh[:], tsh[:], rstd)
                xn = []
                for at in range(AT):
                    xnt = xnpool.tile([P, F], F32R, tag="xn")
                    nc.gpsimd.tensor_scalar(
                        out=xnt[:], in0=x3[:, at, :],
                        scalar1=rstd[:, at:at + 1],
                        scalar2=tsh[:, at:at + 1],
                        op0=mybir.AluOpType.mult,
                        op1=mybir.AluOpType.add)
                    xn.append(xnt)
                xt = []
                for ft in range(KT1):
                    pxt = ps_tp.tile([P, SUP], F32R, space="PSUM", tag="pxt")
                    for at in range(AT):
                        nc.tensor.matmul(
                            out=pxt[:, at * P:(at + 1) * P],
                            lhsT=xn[at][:, ft * P:(ft + 1) * P],
                            rhs=ident[:],
                            is_transpose=True,
                            start=(at == 0), stop=(at == AT - 1),
                        )
                    xtt = xtpool.tile([P, SUP], F32R, tag="xt")
                    if F == F_PS:
                        nc.scalar.copy(out=xtt[:], in_=pxt[:])
                    else:
                        nc.vector.tensor_copy(out=xtt[:], in_=pxt[:])
                    xt.append(xtt)
                h1 = []
                for mt in range(H // P):
                    pg = ps_mm.tile([P, SUP], F32, space="PSUM", tag="pg")
                    for kt in range(KT1):
                        nc.tensor.matmul(
                            out=pg[:],
                            lhsT=w1_t[kt][:, mt * P:(mt + 1) * P],
                            rhs=xt[kt][:],
                            start=(kt == 0), stop=(kt == KT1 - 1),
                        )
                    h1t = hpool.tile([P, SUP], F32R, tag="h1")
                    nc.scalar.activation(out=h1t[:], in_=pg[:],
                                         func=ACTF.Silu,
                                         bias=b1_t[:, mt:mt + 1], scale=1.0)
                    h1.append(h1t)
                h2 = []
                for mt in range(H // P):
                    pg = ps_mm.tile([P, SUP], F32, space="PSUM", tag="pg")
                    for kt in range(H // P):
                        nc.tensor.matmul(
                            out=pg[:],
                            lhsT=w2_t[kt][:, mt * P:(mt + 1) * P],
                            rhs=h1[kt][:],
                            start=(kt == 0), stop=(kt == H // P - 1),
                        )
                    h2t = hpool.tile([P, SUP], F32R, tag="h2")
                    nc.scalar.activation(out=h2t[:], in_=pg[:],
                                         func=ACTF.Silu, scale=1.0)
                    h2.append(h2t)
                for kt in range(H // P):
                    nc.tensor.matmul(
                        out=pe[:],
                        lhsT=w3_t[:, kt:kt + 1],
                        rhs=h2[kt][:],
                        start=(first_l3 and kt == 0),
                        stop=(last_l3 and kt == H // P - 1),
                    )

        CPS = SUP // P  # segsum chunks per supertile
        seg_starts = _seg_starts()

        for _rep in range(nrep):
            ps_seg = ps_sg.tile([1, N_MOL], F32, space="PSUM", tag="pseg")
            for s in range(NSUP):
                # e_ps and e_mp L3s accumulate into one PSUM tile [1, SUP]
                pe = ps_e.tile([1, SUP], F32, space="PSUM", tag="pe")
                stream_sup(xps_d, F_PS, wps1_t, bps1_t, wps2_t, wps3_t,
                           s, pe, True, False)
                stream_sup(xmp_d, F_MP, wmp1_t, bmp1_t, wmp2_t, wmp3_t,
                           s, pe, False, True)
                e_sb = erow_pool.tile([1, SUP], F32, tag="e_sb")
                nc.vector.tensor_copy(out=e_sb[:], in_=pe[:])
                e_dram = dram.tile([SUP], F32, tag="e_dram")
                nc.sync.dma_start(out=e_dram[:], in_=e_sb[:])
                e_cols = erow_pool.tile([P, CPS], mybir.dt.float32r,
                                        tag="e_cols")
                nc.gpsimd.dma_start(
                    out=e_cols[:],
                    in_=e_dram[:].rearrange("(c p) -> p c", p=P),
                )
                for cc in range(CPS):
                    ch = s * CPS + cc
                    st, wid = seg_starts[ch]
                    oh = onepool.tile([P, N_MOL], mybir.dt.float32r,
                                      tag="oh")
                    nc.vector.tensor_scalar(
                        out=oh[:, :wid], in0=iota_t[:, :wid],
                        scalar1=bcol_t[:, ch:ch + 1],
                        scalar2=None, op0=mybir.AluOpType.is_equal,
                    )
                    nc.tensor.matmul(
                        out=ps_seg[0:1, st:st + wid],
                        lhsT=e_cols[:, cc:cc + 1], rhs=oh[:, :wid],
                        start=(ch == 0), stop=(ch == NCHUNK - 1),
                    )
            out_sb = erow_pool.tile([1, N_MOL], F32, tag="out_sb")
            nc.vector.tensor_copy(out=out_sb[:], in_=ps_seg[:])
            nc.sync.dma_start(out=out_d[:], in_=out_sb[:])

    nc.compile()
    _CACHE[("nc", nrep)] = nc
    return nc


def _shard_inputs(x_ps, x_mp, batch, gamma_ps, beta_ps, gamma_mp, beta_mp,
                  W_ps1, W_ps2, W_ps3, W_mp1, W_mp2, W_mp3, nrep=1):
    f32 = np.float32
    cachebust = np.zeros((nrep, _src_tag()), f32)
    x_ps = np.asarray(x_ps, dtype=f32)
    x_mp = np.asarray(x_mp, dtype=f32)
    batch = np.asarray(batch).astype(np.int64)
    sw = _seg_starts()
    starts = np.array([s for s, _ in sw])
    widths = np.array([w for _, w in sw])
    blk = N_ATOMS // N_SPECIES
    in_maps = []
    for c in range(N_CORES):
        s = c // (N_CORES // N_SPECIES)
        h = c % (N_CORES // N_SPECIES)
        bb = batch[s * blk:(s + 1) * blk]
        perm = np.argsort(bb, kind="stable")[h * APC:(h + 1) * APC]
        gidx = s * blk + perm
        bs = bb[perm]
        # local molecule coords: shift by -256*h +64 guard, then per-chunk
        # window start subtraction (window membership asserted below)
        shifted = bs - (N_MOL // 2) * h + 64
        bc = shifted.reshape(NCHUNK, P) - starts[:, None]
        assert (bc >= 0).all() and (bc < widths[:, None]).all(), \
            "segment window overflow - pathological batch distribution"
        w1p = (np.asarray(gamma_ps, f32)[:, None] * np.asarray(W_ps1[s], f32))
        b1p = (np.asarray(beta_ps, f32) @ np.asarray(W_ps1[s], f32))
        w1m = (np.asarray(gamma_mp, f32)[:, None] * np.asarray(W_mp1[s], f32))
        b1m = (np.asarray(beta_mp, f32) @ np.asarray(W_mp1[s], f32))
        in_maps.append({
            "cachebust": cachebust,
            "xps": np.ascontiguousarray(x_ps[gidx]),
            "xmp": np.ascontiguousarray(x_mp[gidx]),
            "bcol": np.ascontiguousarray(bc.T.astype(f32)),
            "wps1": np.ascontiguousarray(w1p.astype(f32)),
            "bps1": np.ascontiguousarray(b1p.astype(f32).reshape(H // P, P).T),
            "wps2": np.ascontiguousarray(np.asarray(W_ps2[s], dtype=f32)),
            "wps3": np.ascontiguousarray(
                np.asarray(W_ps3[s], dtype=f32)[:, 0].reshape(H // P, P).T),
            "wmp1": np.ascontiguousarray(w1m.astype(f32)),
            "bmp1": np.ascontiguousarray(b1m.astype(f32).reshape(H // P, P).T),
            "wmp2": np.ascontiguousarray(np.asarray(W_mp2[s], dtype=f32)),
            "wmp3": np.ascontiguousarray(
                np.asarray(W_mp3[s], dtype=f32)[:, 0].reshape(H // P, P).T),
        })
    return in_maps


def _gather_output(partials):
    """Sum per-core partial energies, undoing each core's local molecule
    coordinate shift (local j corresponds to global m = j + 256*h - 64)."""
    full = np.zeros(N_MOL, dtype=np.float64)
    for c, part in enumerate(partials):
        h = c % (N_CORES // N_SPECIES)
        off = (N_MOL // 2) * h - 64
        j = np.arange(N_MOL)
        m = j + off
        valid = (m >= 0) & (m < N_MOL)
        np.add.at(full, m[valid], part.astype(np.float64)[valid])
    return full.astype(np.float32)


def kernel(x_ps, x_mp, batch, gamma_ps, beta_ps, gamma_mp, beta_mp,
           W_ps1, W_ps2, W_ps3, W_mp1, W_mp2, W_mp3, _want_results=False):
    from concourse.bass_utils import run_bass_kernel_spmd

    nc = _build()
    in_maps = _shard_inputs(
        x_ps, x_mp, batch, gamma_ps, beta_ps, gamma_mp, beta_mp,
        W_ps1, W_ps2, W_ps3, W_mp1, W_mp2, W_mp3)
    res = run_bass_kernel_spmd(nc, in_maps, list(range(N_CORES)))
    partials = [res.results[c]["out"] for c in range(N_CORES)]
    out = _gather_output(partials).reshape(N_MOL, 1)
    if _want_results:
        return out, res
    return out
